# revision 3
# baseline (speedup 1.0000x reference)
import sys

sys.path.insert(0, "/opt/trn_rl_repo")
import numpy as np

import concourse.bacc as bacc
import concourse.mybir as mybir
import concourse.tile as tile
from concourse import bass_utils
from concourse._compat import axon_active
from concourse.masks import make_identity

f32 = mybir.dt.float32
f32r = mybir.dt.float32r

B, H, W, C = 4, 64, 64, 512
N = H * W          # 4096 rows per batch
NOWN = N // 2      # 2048 rows owned per core
D = 64             # qk head dim
NCORES = 8
MARGIN = 30.0      # sampled-max safety margin

TRACE = False
LAST_EXEC_NS = None

_CACHE = {}


def _build(gamma_f, rep=1):
    nc = bacc.Bacc(
        "TRN2", target_bir_lowering=False, debug=not axon_active(), num_devices=1
    )
    xT_d = nc.dram_tensor("xT", [C, N], f32, kind="ExternalInput").ap()
    wq_d = nc.dram_tensor("Wq", [128, 4 * D], f32, kind="ExternalInput").ap()
    wk_d = nc.dram_tensor("Wk", [128, 4 * D], f32, kind="ExternalInput").ap()
    wv_d = nc.dram_tensor("Wv", [128, 4 * C], f32, kind="ExternalInput").ap()
    out_d = nc.dram_tensor("out", [NOWN, C], f32, kind="ExternalOutput").ap()

    X = mybir.AxisListType.X
    MUL = mybir.AluOpType.mult
    MAX = mybir.AluOpType.max
    EXP = mybir.ActivationFunctionType.Exp
    IDn = mybir.ActivationFunctionType.Identity

    with tile.TileContext(nc) as tc:
        with tc.tile_pool(name="sb", bufs=1) as pool, tc.tile_pool(
            name="ps", bufs=1, space="PSUM"
        ) as psum:
            wq_sb = pool.tile([128, 4 * D], f32)
            wk_sb = pool.tile([128, 4 * D], f32)
            wv_sb = pool.tile([128, 4 * C], f32)
            nc.sync.dma_start(wq_sb, wq_d)
            nc.sync.dma_start(wk_sb, wk_d)
            nc.sync.dma_start(wv_sb, wv_d)

            xT = [pool.tile([128, N], f32, name=f"xT{i}") for i in range(4)]
            qT = pool.tile([65, N], f32)       # rows 0..63 = q.T, row 64 = ones
            kT = pool.tile([65, NOWN], f32)    # rows 0..63 = k.T, row 64 = -(smax+m)
            qs = pool.tile([64, 256], f32)     # sampled q columns
            v_sb = [pool.tile([128, C], f32, name=f"v{i}") for i in range(32)]
            ones128 = pool.tile([128, 1], f32)
            onesK1 = pool.tile([1, 1], f32)
            mbias128 = pool.tile([128, 1], f32)
            ident = pool.tile([128, 128], f32)
            make_identity(nc, ident)
            nc.vector.memset(ones128, 1.0)
            nc.vector.memset(onesK1, 1.0)
            nc.vector.memset(mbias128, -MARGIN)
            nc.vector.memset(qT[D : D + 1, :], 1.0)

            for r in range(rep):
                # ---- A: load pre-transposed x ----
                for cb in range(4):
                    nc.sync.dma_start(xT[cb], xT_d[cb * 128 : (cb + 1) * 128, :])

                # ---- B: q (all rows), k (own rows) projections ----
                for ch in range(8):
                    pq = psum.tile([128, 512], f32, tag="work", bufs=2)
                    for cb in range(4):
                        nc.tensor.matmul(
                            pq[0:D, :],
                            wq_sb[:, cb * D : (cb + 1) * D],
                            xT[cb][:, ch * 512 : (ch + 1) * 512],
                            start=(cb == 0), stop=(cb == 3),
                        )
                    nc.scalar.copy(qT[0:D, ch * 512 : (ch + 1) * 512], pq[0:D, :])
                for ch in range(4):
                    pk = psum.tile([128, 512], f32, tag="work", bufs=2)
                    for cb in range(4):
                        nc.tensor.matmul(
                            pk[0:D, :],
                            wk_sb[:, cb * D : (cb + 1) * D],
                            xT[cb][:, ch * 512 : (ch + 1) * 512],
                            start=(cb == 0), stop=(cb == 3),
                        )
                    nc.scalar.copy(kT[0:D, ch * 512 : (ch + 1) * 512], pk[0:D, :])

                # ---- C': sampled row-max -> kT row 64 = -(smax + MARGIN) ----
                nc.vector.tensor_copy(qs, qT[0:D, 0 : N : 16])
                negmax = pool.tile([128, 16], f32, tag="negmax", bufs=2)
                for ib in range(16):
                    es = psum.tile([128, 512], f32, tag="work", bufs=2)
                    nc.tensor.matmul(
                        es[:, 0:256], kT[0:D, ib * 128 : (ib + 1) * 128], qs,
                        start=True, stop=True,
                    )
                    nc.vector.reduce_max(
                        negmax[:, ib : ib + 1], es[:, 0:256], axis=X, negate=True
                    )
                nm2 = pool.tile([128, 16], f32, tag="nm2", bufs=2)
                nc.scalar.activation(nm2, negmax, IDn, bias=mbias128[:, 0:1])
                for ic in range(4):
                    pz = psum.tile([1, 512], f32, tag="accz", bufs=2,
                                   name=f"pz{r}_{ic}")
                    for t in range(4):
                        nc.tensor.transpose(
                            pz[0:1, t * 128 : (t + 1) * 128],
                            nm2[:, ic * 4 + t : ic * 4 + t + 1],
                            ident,
                        )
                    nc.scalar.copy(
                        kT[D : D + 1, ic * 512 : (ic + 1) * 512], pz
                    )

                # ---- D: v projection (all rows) ----
                for jb in range(32):
                    pv = psum.tile([128, 512], f32, tag="work", bufs=2)
                    for cb in range(4):
                        nc.tensor.matmul(
                            pv,
                            xT[cb][:, jb * 128 : (jb + 1) * 128],
                            wv_sb[:, cb * C : (cb + 1) * C],
                            start=(cb == 0), stop=(cb == 3),
                        )
                    nc.scalar.copy(v_sb[jb], pv)

                # ---- E: two-pass-free flash attention over own i rows ----
                for it in range(4):
                    accv = [
                        psum.tile([128, C], f32, tag="accv", bufs=4,
                                  name=f"accv{r}_{it}_{s}")
                        for s in range(4)
                    ]
                    accz = psum.tile([1, 512], f32, tag="accz", bufs=2,
                                     name=f"accz{r}_{it}")
                    for jc in range(32):
                        eps = psum.tile([128, 512], f32, tag="work", bufs=2)
                        nc.tensor.matmul(
                            eps,
                            qT[:, jc * 128 : (jc + 1) * 128],
                            kT[:, it * 512 : (it + 1) * 512],
                            start=True, stop=True,
                        )
                        st = pool.tile([128, 512], f32, tag="st", bufs=3)
                        nc.scalar.activation(st, eps, EXP)
                        for s in range(4):
                            nc.tensor.matmul(
                                accv[s],
                                st[:, s * 128 : (s + 1) * 128],
                                v_sb[jc],
                                start=(jc == 0), stop=(jc == 31),
                            )
                        nc.tensor.matmul(
                            accz,
                            ones128,
                            st,
                            start=(jc == 0), stop=(jc == 31),
                        )
                    z_sb = pool.tile([1, 512], f32, tag="z", bufs=2)
                    nc.scalar.copy(z_sb, accz)
                    zTp = psum.tile([128, 512], f32, tag="work", bufs=2)
                    for s in range(4):
                        nc.tensor.matmul(
                            zTp[:, s : s + 1],
                            z_sb[0:1, s * 128 : (s + 1) * 128],
                            onesK1,
                            start=True, stop=True,
                        )
                    rec = pool.tile([128, 4], f32, tag="rec", bufs=2)
                    nc.vector.reciprocal(rec, zTp[:, 0:4])
                    recg = pool.tile([128, 4], f32, tag="recg", bufs=2)
                    nc.vector.tensor_scalar_mul(recg, rec, gamma_f)
                    for s in range(4):
                        ob = pool.tile([128, C], f32, tag="ob", bufs=3)
                        nc.scalar.activation(
                            ob, accv[s], mybir.ActivationFunctionType.Copy,
                            scale=recg[:, s : s + 1],
                        )
                        nc.sync.dma_start(
                            out_d[it * 512 + s * 128 : it * 512 + (s + 1) * 128, :],
                            ob,
                        )

    nc.compile()
    return nc


def _in_maps(x, Wq, Wk, Wv):
    wq = np.asarray(Wq, dtype=np.float32).reshape(4, 128, D).transpose(1, 0, 2)
    wq = np.ascontiguousarray(wq.reshape(128, 4 * D))
    wk = np.asarray(Wk, dtype=np.float32).reshape(4, 128, D).transpose(1, 0, 2)
    wk = np.ascontiguousarray(wk.reshape(128, 4 * D))
    wv = np.asarray(Wv, dtype=np.float32).reshape(4, 128, C).transpose(1, 0, 2)
    wv = np.ascontiguousarray(wv.reshape(128, 4 * C))
    maps = []
    for c in range(NCORES):
        b, h = c // 2, c % 2
        xb = np.asarray(x[b], dtype=np.float32).reshape(N, C)
        xr = np.roll(xb, -h * NOWN, axis=0)
        xt = np.ascontiguousarray(xr.T)
        maps.append({"xT": xt, "Wq": wq, "Wk": wk, "Wv": wv})
    return maps


def _gather(results):
    out = np.empty((B, N, C), dtype=np.float32)
    for c in range(NCORES):
        b, h = c // 2, c % 2
        out[b, h * NOWN : (h + 1) * NOWN, :] = results[c]["out"]
    return out.reshape(B, H, W, C)


def kernel(x, Wq, Wk, Wv, gamma):
    global LAST_EXEC_NS
    gamma_f = float(np.asarray(gamma).reshape(-1)[0])
    nc = _CACHE.get(gamma_f)
    if nc is None:
        nc = _build(gamma_f)
        _CACHE[gamma_f] = nc

    res = bass_utils.run_bass_kernel_spmd(
        nc, _in_maps(x, Wq, Wk, Wv), core_ids=list(range(NCORES)), trace=TRACE
    )
    LAST_EXEC_NS = getattr(res, "exec_time_ns", None)
    return _gather(res.results)
